# revision 15
# baseline (speedup 1.0000x reference)
"""2D Haar DWT (single level) on Trainium2, 8 NeuronCores, pure data parallel.

Math: per-2x2-block butterflies (ll,lh,hl,hh) = 0.5*(x00 +/- x01 +/- x10
+/- x11).  bf16 crosses HBM both ways (host casts; *0.5 folded into the
PSUM evacuation): 4 MiB in + 4 MiB out per core.

The WHOLE transform is one matmul per 512-column chunk: the host puts
(row parity rp, column parity t) on the PARTITION axis —
p = rp*64 + t*32 + mm, image row = 2*m+rp with m = 32*g8+mm,
col = 2k+t — so each output element is
a +/-1 combination of 4 partitions with equal (mm): a single stationary
matrix B4[128,128] with B4[rp*64+t*32+mm, (2c+h)*32+mm] = s_c[rp]*s_h[t]
(s_0=[1,1], s_1=[1,-1]) computes ALL FOUR subbands at once:
PSUM[(2c+h)*32+mm][g8][k][j] = unscaled subband value, accumulated
exactly in f32.  PE (idle otherwise, 2.4 GHz) streams X through B4;
DVE and ACT split the PSUM -> bf16 SBUF evacuation with the 0.5 scale
folded in.  No width/height tensor ops at all, and only ONE bf16
rounding of the result (better precision than a two-pass butterfly).

Out DRAM is written in SBUF-native order (4-8 KiB runs); the host
un-permutes.  Units are g8-ranges [1,2,2,2,1] (0.5 MiB per g8, 4 KiB
in-runs) — tapered so the first out-DMA starts early.  In-DMAs on the
SP HWDGE ring, out-DMAs on the ACT ring.
"""

import numpy as np
import ml_dtypes

import concourse.mybir as mybir
from concourse import bacc, tile
from concourse.bass_utils import run_bass_kernel_spmd

N_CORES = 8
BATCH = 64
B_PER = BATCH // N_CORES  # 8 images per core
H = W = 512

BF16 = ml_dtypes.bfloat16
UNITS = [(0, 1), (1, 2), (3, 2), (5, 2), (7, 1)]  # (g8 start, n g8-blocks)

_nc_cache = None


def build_bass():
    bf16 = mybir.dt.bfloat16
    f32 = mybir.dt.float32
    nc = bacc.Bacc(
        "TRN2", target_bir_lowering=False, debug=False, num_devices=N_CORES
    )
    # [p = rp*64+t*32+mm][g8][k][j]
    inp = nc.dram_tensor(
        "inputs", [128, 8, 256, 8], bf16, kind="ExternalInput"
    ).ap()
    bmat = nc.dram_tensor("bmat", [128, 128], bf16, kind="ExternalInput").ap()
    # [p' = (2c+h)*32+mm][g8][k][j]
    out = nc.dram_tensor("out", [128, 16384], bf16, kind="ExternalOutput").ap()

    with tile.TileContext(nc) as tc:
        pool_cm = tc.tile_pool(name="p", bufs=3)
        pool = pool_cm.__enter__()
        ps_cm = tc.psum_pool(name="ps", bufs=4)
        psp = ps_cm.__enter__()

        lp_cm = nc.allow_low_precision(reason="bf16 DWT: rel-err budget 2e-2")
        lp_cm.__enter__()

        B = pool.tile([128, 128], bf16, tag="B", bufs=1)
        nc.sync.dma_start(out=B[:], in_=bmat[:])

        chunk_idx = [0]

        def unit(g0, ng, off, k0=0, ks=256, split_in=False):
            F = 8 * ks * ng
            X = pool.tile([128, F], bf16, tag="X", bufs=3)
            if split_in:
                # two half-DMAs so the first matmul can start sooner
                nc.sync.dma_start(
                    out=X[:, 0 : F // 2],
                    in_=inp[:, g0 : g0 + ng, k0 : k0 + ks // 2, :],
                )
                nc.sync.dma_start(
                    out=X[:, F // 2 : F],
                    in_=inp[:, g0 : g0 + ng, k0 + ks // 2 : k0 + ks, :],
                )
            else:
                nc.sync.dma_start(
                    out=X[:], in_=inp[:, g0 : g0 + ng, k0 : k0 + ks, :]
                )
            Yb = pool.tile([128, F], bf16, tag="Yb", bufs=3)
            for c0 in range(0, F, 1024):
                ps = psp.tile([128, 1024], f32, tag="ps")
                nc.tensor.matmul(ps[:, 0:512], B[:], X[:, c0 : c0 + 512])
                nc.tensor.matmul(ps[:, 512:1024], B[:], X[:, c0 + 512 : c0 + 1024])
                dst = Yb[:, c0 : c0 + 1024]
                if chunk_idx[0] % 2 == 0:
                    nc.vector.tensor_scalar_mul(dst, ps[:], 0.5)
                else:
                    nc.scalar.mul(dst, ps[:], 0.5)
                chunk_idx[0] += 1
            nc.scalar.dma_start(out=out[:, off : off + F], in_=Yb[:])

        # g8-range units, tapered; last g8 split into two k-halves so the
        # tail chain (compute+out of the final unit) is short
        unit(0, 1, 0, split_in=True)
        unit(1, 2, 2048)
        unit(3, 2, 6144)
        unit(5, 2, 10240)
        unit(7, 1, 14336, k0=0, ks=128)
        unit(7, 1, 15360, k0=128, ks=128)

        lp_cm.__exit__(None, None, None)
        ps_cm.__exit__(None, None, None)
        pool_cm.__exit__(None, None, None)

    nc.compile()
    return nc


def make_bmat():
    b = np.zeros((128, 128), dtype=np.float32)
    mm = np.arange(32)
    sgn = [np.array([1.0, 1.0]), np.array([1.0, -1.0])]
    for rp in range(2):
        for t in range(2):
            for c in range(2):
                for h in range(2):
                    b[rp * 64 + t * 32 + mm, (2 * c + h) * 32 + mm] = (
                        sgn[c][rp] * sgn[h][t]
                    )
    return b.astype(BF16)


def prep_inputs(x):
    """x: (64, 512, 512) f32 -> per-core [128, 8, 256, 8] bf16."""
    # [B][g8][mm][rp][k][t]: row = 2*(32*g8+mm)+rp, col = 2k+t
    arr = np.asarray(x, dtype=np.float32).reshape(BATCH, 8, 32, 2, 256, 2)
    arr = arr.astype(BF16)
    shards = []
    for c in range(N_CORES):
        blk = arr[c * B_PER : (c + 1) * B_PER]  # [j][g8][mm][rp][k][t]
        blk = blk.transpose(3, 5, 2, 1, 4, 0)  # [rp][t][mm][g8][k][j]
        shards.append(np.ascontiguousarray(blk).reshape(128, 8, 256, 8))
    return shards


def assemble_output(outs):
    """outs: per-core [128, 16384] bf16 -> (64, 512, 512, 1) f32 (scaled)."""
    res = np.empty((BATCH, H, W), dtype=np.float32)
    for core, o in enumerate(outs):
        # [c][h][mm][g8][k][j] -> [j][c][g8][mm][h][k]
        blk = o.reshape(2, 2, 32, 8, 256, 8).transpose(5, 0, 3, 2, 1, 4)
        res[core * B_PER : (core + 1) * B_PER] = blk.reshape(B_PER, H, W)
    return res.reshape(BATCH, H, W, 1)


def kernel(**inputs):
    global _nc_cache
    x = np.asarray(inputs["inputs"], dtype=np.float32).reshape(BATCH, H, W)
    shards = prep_inputs(x)
    bm = make_bmat()
    if _nc_cache is None:
        _nc_cache = build_bass()
    nc = _nc_cache
    in_maps = [{"inputs": shards[i], "bmat": bm} for i in range(N_CORES)]
    res = run_bass_kernel_spmd(nc, in_maps, core_ids=list(range(N_CORES))).results
    return assemble_output([res[i]["out"] for i in range(N_CORES)])


# revision 18
# speedup vs baseline: 1.0361x; 1.0361x over previous
"""2D Haar DWT (single level) on Trainium2, 8 NeuronCores, pure data parallel.

Math: per-2x2-block butterflies (ll,lh,hl,hh) = 0.5*(x00 +/- x01 +/- x10
+/- x11).  bf16 crosses HBM both ways (host casts; *0.5 folded into the
PSUM evacuation): 4 MiB in + 4 MiB out per core.

The WHOLE transform is one matmul per 512-column chunk: the host puts
(row parity rp, column parity t) on the PARTITION axis —
p = rp*64 + t*32 + mm, image row = 2*m+rp with m = 32*g8+mm,
col = 2k+t — so each output element is
a +/-1 combination of 4 partitions with equal (mm): a single stationary
matrix B4[128,128] with B4[rp*64+t*32+mm, (2c+h)*32+mm] = s_c[rp]*s_h[t]
(s_0=[1,1], s_1=[1,-1]) computes ALL FOUR subbands at once:
PSUM[(2c+h)*32+mm][g8][k][j] = unscaled subband value, accumulated
exactly in f32.  PE (idle otherwise, 2.4 GHz) streams X through B4;
DVE and ACT split the PSUM -> bf16 SBUF evacuation with the 0.5 scale
folded in.  No width/height tensor ops at all, and only ONE bf16
rounding of the result (better precision than a two-pass butterfly).

Out DRAM is written in SBUF-native order (4-8 KiB runs); the host
un-permutes.  Units are g8-ranges [1,2,2,2,1] (0.5 MiB per g8, 4 KiB
in-runs) — tapered so the first out-DMA starts early.  In-DMAs on the
SP HWDGE ring, out-DMAs on the ACT ring.
"""

import numpy as np
import ml_dtypes

import concourse.mybir as mybir
from concourse import bacc, tile
from concourse.bass_utils import run_bass_kernel_spmd

N_CORES = 8
BATCH = 64
B_PER = BATCH // N_CORES  # 8 images per core
H = W = 512

BF16 = ml_dtypes.bfloat16
UNITS = [(0, 1), (1, 2), (3, 2), (5, 2), (7, 1)]  # (g8 start, n g8-blocks)

_nc_cache = None


def build_bass():
    bf16 = mybir.dt.bfloat16
    f32 = mybir.dt.float32
    nc = bacc.Bacc(
        "TRN2", target_bir_lowering=False, debug=False, num_devices=N_CORES
    )
    # [p = rp*64+t*32+mm][g8][k][j]
    inp = nc.dram_tensor(
        "inputs", [128, 8, 256, 8], bf16, kind="ExternalInput"
    ).ap()
    bmat = nc.dram_tensor("bmat", [128, 128], bf16, kind="ExternalInput").ap()
    # [p' = (2c+h)*32+mm][g8][k][j]
    out = nc.dram_tensor("out", [128, 16384], bf16, kind="ExternalOutput").ap()

    with tile.TileContext(nc) as tc:
        pool_cm = tc.tile_pool(name="p", bufs=3)
        pool = pool_cm.__enter__()
        ps_cm = tc.psum_pool(name="ps", bufs=4)
        psp = ps_cm.__enter__()

        lp_cm = nc.allow_low_precision(reason="bf16 DWT: rel-err budget 2e-2")
        lp_cm.__enter__()

        B = pool.tile([128, 128], bf16, tag="B", bufs=1)
        nc.sync.dma_start(out=B[:], in_=bmat[:])

        chunk_idx = [0]

        def unit(g0, ng, off, k0=0, ks=256, split_in=False):
            F = 8 * ks * ng
            X = pool.tile([128, F], bf16, tag="X", bufs=4)
            if split_in:
                # two half-DMAs so the first matmul can start sooner
                nc.sync.dma_start(
                    out=X[:, 0 : F // 2],
                    in_=inp[:, g0 : g0 + ng, k0 : k0 + ks // 2, :],
                )
                nc.sync.dma_start(
                    out=X[:, F // 2 : F],
                    in_=inp[:, g0 : g0 + ng, k0 + ks // 2 : k0 + ks, :],
                )
            else:
                nc.sync.dma_start(
                    out=X[:], in_=inp[:, g0 : g0 + ng, k0 : k0 + ks, :]
                )
            Yb = pool.tile([128, F], bf16, tag="Yb", bufs=4)
            for c0 in range(0, F, 1024):
                ps = psp.tile([128, 1024], f32, tag="ps")
                nc.tensor.matmul(ps[:, 0:512], B[:], X[:, c0 : c0 + 512])
                nc.tensor.matmul(ps[:, 512:1024], B[:], X[:, c0 + 512 : c0 + 1024])
                dst = Yb[:, c0 : c0 + 1024]
                if chunk_idx[0] % 2 == 0:
                    nc.vector.tensor_scalar_mul(dst, ps[:], 0.5)
                else:
                    nc.scalar.mul(dst, ps[:], 0.5)
                chunk_idx[0] += 1
            nc.scalar.dma_start(out=out[:, off : off + F], in_=Yb[:])

        # g8-range units, tapered; first in-DMA split so PE starts early
        unit(0, 1, 0, split_in=True)
        unit(1, 2, 2048)
        unit(3, 2, 6144)
        unit(5, 2, 10240)
        unit(7, 1, 14336)

        lp_cm.__exit__(None, None, None)
        ps_cm.__exit__(None, None, None)
        pool_cm.__exit__(None, None, None)

    nc.compile()
    return nc


def make_bmat():
    b = np.zeros((128, 128), dtype=np.float32)
    mm = np.arange(32)
    sgn = [np.array([1.0, 1.0]), np.array([1.0, -1.0])]
    for rp in range(2):
        for t in range(2):
            for c in range(2):
                for h in range(2):
                    b[rp * 64 + t * 32 + mm, (2 * c + h) * 32 + mm] = (
                        sgn[c][rp] * sgn[h][t]
                    )
    return b.astype(BF16)


def prep_inputs(x):
    """x: (64, 512, 512) f32 -> per-core [128, 8, 256, 8] bf16."""
    # [B][g8][mm][rp][k][t]: row = 2*(32*g8+mm)+rp, col = 2k+t
    arr = np.asarray(x, dtype=np.float32).reshape(BATCH, 8, 32, 2, 256, 2)
    arr = arr.astype(BF16)
    shards = []
    for c in range(N_CORES):
        blk = arr[c * B_PER : (c + 1) * B_PER]  # [j][g8][mm][rp][k][t]
        blk = blk.transpose(3, 5, 2, 1, 4, 0)  # [rp][t][mm][g8][k][j]
        shards.append(np.ascontiguousarray(blk).reshape(128, 8, 256, 8))
    return shards


def assemble_output(outs):
    """outs: per-core [128, 16384] bf16 -> (64, 512, 512, 1) f32 (scaled)."""
    res = np.empty((BATCH, H, W), dtype=np.float32)
    for core, o in enumerate(outs):
        # [c][h][mm][g8][k][j] -> [j][c][g8][mm][h][k]
        blk = o.reshape(2, 2, 32, 8, 256, 8).transpose(5, 0, 3, 2, 1, 4)
        res[core * B_PER : (core + 1) * B_PER] = blk.reshape(B_PER, H, W)
    return res.reshape(BATCH, H, W, 1)


def kernel(**inputs):
    global _nc_cache
    x = np.asarray(inputs["inputs"], dtype=np.float32).reshape(BATCH, H, W)
    shards = prep_inputs(x)
    bm = make_bmat()
    if _nc_cache is None:
        _nc_cache = build_bass()
    nc = _nc_cache
    in_maps = [{"inputs": shards[i], "bmat": bm} for i in range(N_CORES)]
    res = run_bass_kernel_spmd(nc, in_maps, core_ids=list(range(N_CORES))).results
    return assemble_output([res[i]["out"] for i in range(N_CORES)])
